# revision 7
# baseline (speedup 1.0000x reference)
"""Causal self-attention (B=2, T=2048, C=1024, H=16) on 8 TRN2 NeuronCores.

Sharding: core = (batch b, head-group hg) with b in {0,1}, hg in {0..3};
each core computes Q/K/V projections and attention for its 4 heads on its
batch, plus the row-parallel slice of the output projection. The host sums
the 4 per-core partial projections per batch and adds the output bias.

v3 layout (all transposed so softmax needs no on-chip transposes):
  - Q^T, K^T [dd, t] via 8 parallel c-outer PSUM chains (no head-of-line
    blocking on the streaming xT DMA); V [t, dd] via per-t-chunk chains.
    QKV weights arrive packed as one [C, 3*DD] tensor (one DMA per chunk).
  - S^T[s, t]: 2 heads row-packed (K=64 at row offsets 0/64, concurrent
    streams); diagonal blocks trimmed to cols >= r*P; causal masking via an
    additive 128x128 band.
  - exp on ScalarE straight out of PSUM (scale + optional kbias folded in).
  - softmax denominators from an all-ones column appended to V (M=65 AV
    matmul); normalization deferred to after AV.
  - AV emission runs one j behind S so the PE never head-of-line blocks on
    the exp; independent filler work (V chains, second-half QK chains,
    output-projection tiles) is interleaved between attention slots to keep
    the PE dense (max p-state = 2.4 GHz needs ~3us of continuous work).
  - epilogue per (i,g): PSUM->SBUF copies + a DMA that respreads the
    denominator row to [8,256] (at block end), then deferred into the next
    block: Ln+Exp(-x) on [8,256] (cheap: ACT cost tracks free size), DRAM
    bounce broadcast in bf16, two DVE muls. ScalarE's exp stream never
    waits on an epilogue.
  - y_partial[t, e] accumulated over the 2 dd-chunks, written back as bf16
    on the GPSIMD software-DGE queue (keeps the latency-critical softmax
    bounce DMAs alone on the sync ring); host sums partials in f32.
"""

import math
from functools import lru_cache, partial

import ml_dtypes
import numpy as np

import concourse.bass as bass
import concourse.mybir as mybir
from concourse import bacc
import concourse.tile as tile
from concourse import bass_utils

F32 = mybir.dt.float32
BF16 = mybir.dt.bfloat16
EXP = mybir.ActivationFunctionType.Exp
LN = mybir.ActivationFunctionType.Ln

B, T, C, H = 2, 2048, 1024, 16
NCORES = 8
NH = 4            # heads per core
D = C // H        # 64
DD = NH * D       # 256 channels per core
P = 128
TG = 512          # t-group width (matmul moving dim)
NG = T // TG      # 4
NT = T // P       # 16 s-chunks
CCH = C // P      # 8 contraction chunks
NEG = -8.0e6      # pre-scale additive mask; *0.125 = -1e6 like the reference
SCL = 1.0 / math.sqrt(D)
RSP, RSF = 8, (2 * TG) // 8   # denominator respread shape [8, 256]

LAST_RESULTS = None  # BassKernelResults of the most recent run (for test.py)


def build_program(apply_kbias: bool, general_mask: bool) -> bass.Bass:
    nc = bacc.Bacc("TRN2", target_bir_lowering=False, debug=False,
                   enable_asserts=False)

    xT = nc.dram_tensor("xT", [C, T], BF16, kind="ExternalInput").ap()
    wqkvT = nc.dram_tensor("wqkvT", [C, 3 * DD], BF16, kind="ExternalInput").ap()
    wpT = nc.dram_tensor("wpT", [DD, C], BF16, kind="ExternalInput").ap()
    bqk = nc.dram_tensor("bqk", [P, 4], F32, kind="ExternalInput").ap()
    bv_in = nc.dram_tensor("bv_sb", [P, DD], F32, kind="ExternalInput").ap()
    kbias_in = None
    if apply_kbias:
        kbias_in = nc.dram_tensor("kbias", [P, NT], F32, kind="ExternalInput").ap()
    band_in = maskT = None
    if general_mask:
        maskT = nc.dram_tensor("maskT", [T, T], F32, kind="ExternalInput").ap()
    else:
        band_in = nc.dram_tensor("band", [P, P], F32, kind="ExternalInput").ap()
    yp = nc.dram_tensor("yp", [T, C], BF16, kind="ExternalOutput").ap()
    # DRAM scratch used to broadcast softmax reciprocal rows across
    # partitions (DMA from DRAM may use a 0-step partition dim; SBUF may not)
    rcd = nc.dram_tensor("rcd", [8, RSP, RSF], BF16, kind="Internal").ap()

    with tile.TileContext(nc) as tc:
        with tc.tile_pool(name="wts", bufs=1) as wts, \
             tc.tile_pool(name="xtp", bufs=1) as xtp, \
             tc.tile_pool(name="qkv", bufs=1) as qkv, \
             tc.tile_pool(name="otp", bufs=1) as otp, \
             tc.tile_pool(name="ptp", bufs=2) as ptp, \
             tc.tile_pool(name="asb", bufs=2) as asbp, \
             tc.tile_pool(name="rsp", bufs=2) as rsp, \
             tc.tile_pool(name="bcp", bufs=2) as bcp, \
             tc.tile_pool(name="tmp", bufs=2) as tmpp, \
             tc.tile_pool(name="ysb", bufs=2) as ysbp, \
             tc.tile_pool(name="mkp", bufs=2) as mkp, \
             tc.tile_pool(name="stp", bufs=2, space="PSUM") as stp, \
             tc.tile_pool(name="avp", bufs=2, space="PSUM") as avp, \
             tc.tile_pool(name="mmp", bufs=2, space="PSUM") as mmp:

            # Pre-load the ACT table set containing Exp+Ln so the first use
            # doesn't pay a table load mid-pipeline.
            from concourse.hw_specs import get_activation_tables
            tables = get_activation_tables(nc.m.arch)
            combined_id = list(tables).index("natural_log_exp_and_others")
            nc.scalar.add_instruction(mybir.InstLoadActFuncSet(
                name=nc.get_next_instruction_name(), ins=[], outs=[],
                act_func_set_id=combined_id))

            # ---- input DMAs ----
            # sync ring: the phase-A-critical stream (wqkv[c], xt[c] pairs)
            wqkv = [wts.tile([P, 3 * DD], BF16, name=f"wqkv{c}")
                    for c in range(CCH)]
            xt = [xtp.tile([P, T], BF16, name=f"xt{c}") for c in range(CCH)]
            for c in range(CCH):
                nc.sync.dma_start(out=wqkv[c],
                                  in_=wqkvT[c * P:(c + 1) * P, :])
                nc.sync.dma_start(out=xt[c], in_=xT[c * P:(c + 1) * P, :])
            wk = [wqkv[c][:, 0:DD] for c in range(CCH)]
            wq = [wqkv[c][:, DD:2 * DD] for c in range(CCH)]
            wv = [wqkv[c][:, 2 * DD:3 * DD] for c in range(CCH)]
            # scalar ring: everything that isn't needed in the first ~15us
            bqk_t = wts.tile([P, 4], F32, name="bqk_t")
            nc.scalar.dma_start(out=bqk_t, in_=bqk)
            band_t = None
            if band_in is not None:
                band_t = wts.tile([P, P], F32, name="band_t")
                nc.scalar.dma_start(out=band_t, in_=band_in)
            if kbias_in is not None:
                kbias_t = wts.tile([P, NT], F32, name="kbias_t")
                nc.scalar.dma_start(out=kbias_t, in_=kbias_in)
            bv_sb = wts.tile([P, DD], F32, name="bv_t")
            nc.scalar.dma_start(out=bv_sb, in_=bv_in)
            wp = [wts.tile([P, C], BF16, name=f"wp{i}") for i in range(2)]
            for i in range(2):
                nc.scalar.dma_start(out=wp[i], in_=wpT[i * P:(i + 1) * P, :])

            qt = [qkv.tile([P, T], BF16, name=f"qt{i}") for i in range(2)]
            kt = [qkv.tile([P, T], BF16, name=f"kt{i}") for i in range(2)]
            vaug = [qkv.tile([P, NH * (D + 1)], BF16, name=f"vaug{j}")
                    for j in range(NT)]
            ot = [otp.tile([P, T], BF16, name=f"ot{i}") for i in range(2)]

            # ones columns (softmax denominator rows) written once, on the
            # otherwise-idle GPSIMD engine
            for j in range(NT):
                vview = vaug[j].rearrange("p (h x) -> p h x", h=NH)
                nc.gpsimd.memset(vview[:, :, D:D + 1], 1.0)

            # ---- phase A: kt[0]/qt[0], c-outer over 8 parallel chains ----
            # (each xT chunk arrival unblocks one matmul per chain; the PE
            # is never head-of-line blocked behind a not-yet-arrived chunk)
            stA = [stp.tile([P, 2 * TG], F32, name="st", tag="st")
                   for _ in range(2)]
            avA = [avp.tile([P, TG], F32, name="av", tag="av")
                   for _ in range(2)]
            mmA = [mmp.tile([P, TG], F32, name="mm", tag="mm")
                   for _ in range(2)]
            ps8 = [stA[0][:, 0:TG], stA[0][:, TG:2 * TG],
                   stA[1][:, 0:TG], stA[1][:, TG:2 * TG],
                   avA[0], avA[1], mmA[0], mmA[1]]
            specs = [(kt[0], wk, 2, tg) for tg in range(NG)] + \
                    [(qt[0], wq, 0, tg) for tg in range(NG)]
            for c in range(CCH):
                for idx, (dst, w, col, tg) in enumerate(specs):
                    nc.tensor.matmul(
                        ps8[idx],
                        lhsT=(w[c][:, 0:P]),
                        rhs=(xt[c][:, tg * TG:(tg + 1) * TG]),
                        start=(c == 0), stop=(c == CCH - 1))
            # drain order: the chains attn(0,3) needs first come first
            for idx in (0, 7, 1, 2, 3, 4, 5, 6):
                dst, w, col, tg = specs[idx]
                nc.vector.tensor_scalar_add(
                    dst[:, tg * TG:(tg + 1) * TG], ps8[idx],
                    bqk_t[:, col:col + 1])

            # ---- filler units (independent PE work interleaved into the
            # attention slots to cover the S->exp->AV latency) ----
            def v_proj(j):
                ps = mmp.tile([P, TG], F32, name="mm", tag="mm")
                for c in range(CCH):
                    nc.tensor.matmul(
                        ps[:, :DD],
                        lhsT=(xt[c][:, j * P:(j + 1) * P]),
                        rhs=(wv[c]),
                        start=(c == 0), stop=(c == CCH - 1))
                vview = vaug[j].rearrange("p (h x) -> p h x", h=NH)
                nc.vector.tensor_add(
                    vview[:, :, 0:D],
                    ps[:, :DD].rearrange("p (h x) -> p h x", h=NH),
                    bv_sb.rearrange("p (h x) -> p h x", h=NH))

            def qk1_chain(dst, w, col, tg):
                ps = mmp.tile([P, TG], F32, name="mm", tag="mm")
                for c in range(CCH):
                    nc.tensor.matmul(
                        ps,
                        lhsT=(w[c][:, P:2 * P]),
                        rhs=(xt[c][:, tg * TG:(tg + 1) * TG]),
                        start=(c == 0), stop=(c == CCH - 1))
                nc.vector.tensor_scalar_add(
                    dst[:, tg * TG:(tg + 1) * TG], ps, bqk_t[:, col:col + 1])

            def proj_unit(tt, ec, cast_engine=None):
                ps = mmp.tile([P, TG], F32, name="mm", tag="mm")
                for i in range(2):
                    nc.tensor.matmul(
                        ps,
                        lhsT=(ot[i][:, tt * P:(tt + 1) * P]),
                        rhs=(wp[i][:, ec * TG:(ec + 1) * TG]),
                        start=(i == 0), stop=(i == 1))
                ysb = ysbp.tile([P, TG], BF16, name="ysb", tag="ysb")
                if cast_engine == "scalar":
                    nc.scalar.activation(
                        ysb, ps, mybir.ActivationFunctionType.Identity)
                else:
                    nc.vector.tensor_copy(ysb, ps)
                nc.sync.dma_start(
                    out=yp[tt * P:(tt + 1) * P, ec * TG:(ec + 1) * TG],
                    in_=ysb)

            fq = []
            credit = [0.0]

            def pop_f(rate=1.0):
                credit[0] += rate
                while credit[0] >= 1.0:
                    credit[0] -= 1.0
                    if fq:
                        fq.pop(0)()

            # ---- attention ----
            def attn_block(i, g, rate, prev_epi):
                nj = NT if general_mask else 4 * g + 4
                av = [avp.tile([P, TG], F32, name="av", tag="av")
                      for _ in range(2)]

                def emit_av(j, trim, pt):
                    for h in range(2):
                        nc.tensor.matmul(
                            av[h][0:D + 1, trim:TG],
                            lhsT=(vaug[j][:, (2 * i + h) * (D + 1):
                                               (2 * i + h + 1) * (D + 1)]),
                            rhs=(pt[:, h * TG + trim:(h + 1) * TG]),
                            start=(j == 0), stop=(j == nj - 1),
                            skip_group_check=True)

                pend = None
                for j in range(nj):
                    r = j - 4 * g
                    trim = r * P if (r > 0 and not general_mask) else 0
                    st = stp.tile([P, 2 * TG], F32, name="st", tag="st")
                    for h in range(2):
                        nc.tensor.matmul(
                            st[:, h * TG + trim:(h + 1) * TG],
                            lhsT=(kt[i][64 * h:64 * h + 64,
                                             j * P:(j + 1) * P]),
                            rhs=(qt[i][64 * h:64 * h + 64,
                                            g * TG + trim:(g + 1) * TG]),
                            start=True, stop=True,
                            tile_position=(64 * h, 0))
                    if general_mask:
                        mk = mkp.tile([P, TG], F32, name="mk", tag="mk")
                        nc.sync.dma_start(
                            out=mk,
                            in_=maskT[j * P:(j + 1) * P, g * TG:(g + 1) * TG])
                        for h in range(2):
                            nc.vector.tensor_add(
                                st[:, h * TG:(h + 1) * TG],
                                st[:, h * TG:(h + 1) * TG], mk)
                    elif r >= 0:
                        for h in range(2):
                            sl = slice(h * TG + r * P, h * TG + (r + 1) * P)
                            nc.vector.tensor_add(st[:, sl], st[:, sl], band_t)
                    pt = ptp.tile([P, 2 * TG], BF16, name="pt", tag="pt")
                    kb = kbias_t[:, j:j + 1] if apply_kbias else 0.0
                    if trim > 0:
                        for h in range(2):
                            nc.scalar.activation(
                                pt[:, h * TG + trim:(h + 1) * TG],
                                st[:, h * TG + trim:(h + 1) * TG],
                                EXP, bias=kb, scale=SCL)
                    else:
                        nc.scalar.activation(pt, st, EXP, bias=kb, scale=SCL)
                    if pend is not None:
                        emit_av(*pend)
                    pend = (j, trim, pt)
                    if j == 1 and prev_epi is not None:
                        prev_epi()
                        prev_epi = None
                    pop_f(rate)
                emit_av(*pend)
                if prev_epi is not None:  # nj==1 can't happen, but be safe
                    prev_epi()

                # epilogue part a (now): free the PSUM accumulator banks and
                # kick off the denominator respread. Part b (deferred into
                # the next block) does the scalar Ln/Exp + bounce + muls so
                # the exp stream never idles waiting on this block's AVs.
                slot = i * NG + g
                asb = asbp.tile([D + 1, 2 * TG], F32, name="asb", tag="asb")
                nc.vector.tensor_copy(asb[:, 0:TG], av[0][0:D + 1, :])
                nc.vector.tensor_copy(asb[:, TG:2 * TG], av[1][0:D + 1, :])
                rs = rsp.tile([RSP, RSF], F32, name="rs", tag="rs")
                nc.sync.dma_start(out=rs, in_=asb[D:D + 1, :])

                def epi_b():
                    rsb = rsp.tile([RSP, RSF], BF16, name="rsb", tag="rsb")
                    nc.scalar.activation(rs, rs, LN)
                    nc.scalar.activation(rsb, rs, EXP, scale=-1.0)
                    nc.sync.dma_start(out=rcd[slot], in_=rsb)
                    bc = bcp.tile([P, 2 * TG], BF16, name="bc", tag="bc")
                    bcast_src = bass.AP(
                        tensor=rcd.tensor, offset=rcd[slot].offset,
                        ap=[[0, D], [1, 2 * TG]])
                    nc.sync.dma_start(out=bc[0:D, :], in_=bcast_src)
                    nc.vector.tensor_mul(
                        ot[i][0:D, g * TG:(g + 1) * TG],
                        asb[0:D, 0:TG], bc[0:D, 0:TG])
                    tm = tmpp.tile([P, TG], BF16, name="tm", tag="tm")
                    nc.vector.tensor_mul(tm[0:D, :], asb[0:D, TG:2 * TG],
                                         bc[0:D, TG:2 * TG])
                    nc.sync.dma_start(
                        out=ot[i][64:128, g * TG:(g + 1) * TG],
                        in_=tm[0:D, :])

                return epi_b

            # ---- schedule ----
            for j in range(NT):
                fq.append(partial(v_proj, j))
            for tg in range(NG):
                fq.append(partial(qk1_chain, kt[1], wk, 3, tg))
            fq.append(partial(qk1_chain, qt[1], wq, 1, 3))
            for tg in range(3):
                fq.append(partial(qk1_chain, qt[1], wq, 1, tg))

            def with_proj(epi_b, g):
                # proj units of group g may only enter the filler queue once
                # the epilogue writing ot[*] for g has actually been EMITTED
                # (emission order defines the dependency graph)
                def f():
                    epi_b()
                    for tt in range(4 * g, 4 * g + 4):
                        for ec in range(2):
                            fq.append(partial(proj_unit, tt, ec))
                return f

            pop_f(4.0)  # v_proj 0..3 ahead of attn(0,3)'s first AVs
            epi = attn_block(0, 3, 1.0, None)
            epi = attn_block(0, 2, 1.0, epi)
            epi = attn_block(1, 3, 1.0, epi)
            epi = attn_block(1, 2, 0.75, with_proj(epi, 3))
            epi = attn_block(0, 1, 0.55, epi)
            epi = attn_block(1, 1, 0.55, with_proj(epi, 2))
            epi = attn_block(0, 0, 1.0, epi)
            epi = attn_block(1, 0, 1.0, with_proj(epi, 1))
            epi()
            for u, (tt, ec) in enumerate(
                    (tt, ec) for tt in range(4) for ec in range(2)):
                eng = "scalar" if u % 2 else None
                proj_unit(tt, ec, cast_engine=eng)
            while fq:
                pop_f(1.0)

    nc.compile()
    return nc


@lru_cache(maxsize=4)
def _program(apply_kbias: bool, general_mask: bool) -> bass.Bass:
    return build_program(apply_kbias, general_mask)


def _host_prep(inputs):
    x = np.asarray(inputs["x"], np.float32)
    Wq = np.asarray(inputs["Wq"], np.float32)
    bq = np.asarray(inputs["bq"], np.float32)
    Wk = np.asarray(inputs["Wk"], np.float32)
    bk = np.asarray(inputs["bk"], np.float32)
    Wv = np.asarray(inputs["Wv"], np.float32)
    bv = np.asarray(inputs["bv"], np.float32)
    Wp = np.asarray(inputs["Wp"], np.float32)
    attn_mask = np.asarray(inputs["attn_mask"])
    valid = np.asarray(inputs["valid_input_mask"])

    tril = np.tril(np.ones((T, T), attn_mask.dtype))
    causal = all(np.array_equal(attn_mask[b], tril) for b in range(B))
    # folded into the exp's bias (which applies after the 1/sqrt(d) scale)
    kbias_all = (valid.astype(np.float32) - 1.0) * 1e6
    apply_kbias = bool((valid == 0).any())

    band = np.where(np.arange(P)[:, None] <= np.arange(P)[None, :],
                    np.float32(0.0), np.float32(NEG))

    in_maps = []
    for core in range(NCORES):
        b, hg = divmod(core, 4)
        sl = slice(hg * DD, (hg + 1) * DD)
        wqkv = np.concatenate(
            [Wk[sl, :].T, Wq[sl, :].T, Wv[sl, :].T], axis=1)
        m = {
            "xT": np.ascontiguousarray(x[b].T).astype(ml_dtypes.bfloat16),
            "wqkvT": np.ascontiguousarray(wqkv).astype(ml_dtypes.bfloat16),
            "wpT": np.ascontiguousarray(Wp[:, sl].T).astype(ml_dtypes.bfloat16),
            "bqk": np.ascontiguousarray(
                np.stack([bq[sl][:P], bq[sl][P:], bk[sl][:P], bk[sl][P:]], 1)),
            "bv_sb": np.ascontiguousarray(np.tile(bv[sl], (P, 1))),
        }
        if apply_kbias:
            m["kbias"] = np.ascontiguousarray(kbias_all[b].reshape(NT, P).T)
        if not causal:
            m["maskT"] = np.ascontiguousarray(
                (attn_mask[b].T.astype(np.float32) - 1.0) * (-NEG))
        else:
            m["band"] = band
        in_maps.append(m)
    return in_maps, apply_kbias, causal


def _run(inputs, trace=False, trace_cores=None):
    global LAST_RESULTS
    in_maps, apply_kbias, causal = _host_prep(inputs)
    nc = _program(apply_kbias, not causal)
    res = bass_utils.run_bass_kernel_spmd(
        nc, in_maps, core_ids=list(range(NCORES)), trace=trace,
        trace_cores=trace_cores)
    LAST_RESULTS = res

    bp = np.asarray(inputs["bp"], np.float32)
    y = np.zeros((B, T, C), np.float32)
    for core in range(NCORES):
        y[core // 4] += np.asarray(res.results[core]["yp"], np.float32)
    y += bp[None, None, :]
    return y


def kernel(**inputs) -> np.ndarray:
    return _run(inputs)


# revision 10
# speedup vs baseline: 1.0411x; 1.0411x over previous
"""Causal self-attention (B=2, T=2048, C=1024, H=16) on 8 TRN2 NeuronCores.

Sharding: core = (batch b, head-group hg) with b in {0,1}, hg in {0..3};
each core computes Q/K/V projections and attention for its 4 heads on its
batch, plus the row-parallel slice of the output projection. The host sums
the 4 per-core partial projections per batch and adds the output bias.

v3 layout (all transposed so softmax needs no on-chip transposes):
  - Q^T, K^T [dd, t] via 8 parallel c-outer PSUM chains (no head-of-line
    blocking on the streaming xT DMA); V [t, dd] via per-t-chunk chains.
    QKV weights arrive packed as one [C, 3*DD] tensor (one DMA per chunk).
  - S^T[s, t]: 2 heads row-packed (K=64 at row offsets 0/64, concurrent
    streams); diagonal blocks trimmed to cols >= r*P; causal masking via an
    additive 128x128 band.
  - exp on ScalarE straight out of PSUM (scale + optional kbias folded in).
  - softmax denominators from an all-ones column appended to V (M=65 AV
    matmul); normalization deferred to after AV.
  - AV emission runs one j behind S so the PE never head-of-line blocks on
    the exp; independent filler work (V chains, second-half QK chains,
    output-projection tiles) is interleaved between attention slots to keep
    the PE dense (max p-state = 2.4 GHz needs ~3us of continuous work).
  - epilogue per (i,g): PSUM->SBUF copies + a DMA that respreads the
    denominator row to [8,256] (at block end), then deferred into the next
    block: Ln+Exp(-x) on [8,256] (cheap: ACT cost tracks free size), DRAM
    bounce broadcast in bf16, two DVE muls. ScalarE's exp stream never
    waits on an epilogue.
  - y_partial[t, e] accumulated over the 2 dd-chunks, written back as bf16
    on the GPSIMD software-DGE queue (keeps the latency-critical softmax
    bounce DMAs alone on the sync ring); host sums partials in f32.
"""

import math
from functools import lru_cache, partial

import ml_dtypes
import numpy as np

import concourse.bass as bass
import concourse.mybir as mybir
from concourse import bacc
import concourse.tile as tile
from concourse import bass_utils

F32 = mybir.dt.float32
BF16 = mybir.dt.bfloat16
EXP = mybir.ActivationFunctionType.Exp
LN = mybir.ActivationFunctionType.Ln

B, T, C, H = 2, 2048, 1024, 16
NCORES = 8
NH = 4            # heads per core
D = C // H        # 64
DD = NH * D       # 256 channels per core
P = 128
TG = 512          # t-group width (matmul moving dim)
NG = T // TG      # 4
NT = T // P       # 16 s-chunks
CCH = C // P      # 8 contraction chunks
NEG = -8.0e6      # pre-scale additive mask; *0.125 = -1e6 like the reference
SCL = 1.0 / math.sqrt(D)
RSP, RSF = 8, (2 * TG) // 8   # denominator respread shape [8, 256]

LAST_RESULTS = None  # BassKernelResults of the most recent run (for test.py)


def build_program(apply_kbias: bool, general_mask: bool) -> bass.Bass:
    nc = bacc.Bacc("TRN2", target_bir_lowering=False, debug=False,
                   enable_asserts=False)

    xT = nc.dram_tensor("xT", [C, T], BF16, kind="ExternalInput").ap()
    wqkvT = nc.dram_tensor("wqkvT", [C, 3 * DD], BF16, kind="ExternalInput").ap()
    wpT = nc.dram_tensor("wpT", [DD, C], BF16, kind="ExternalInput").ap()
    bqk = nc.dram_tensor("bqk", [P, 4], F32, kind="ExternalInput").ap()
    bv_in = nc.dram_tensor("bv_sb", [P, DD], F32, kind="ExternalInput").ap()
    kbias_in = None
    if apply_kbias:
        kbias_in = nc.dram_tensor("kbias", [P, NT], F32, kind="ExternalInput").ap()
    band_in = maskT = None
    if general_mask:
        maskT = nc.dram_tensor("maskT", [T, T], F32, kind="ExternalInput").ap()
    else:
        band_in = nc.dram_tensor("band", [P, P], F32, kind="ExternalInput").ap()
    yp = nc.dram_tensor("yp", [T, C], BF16, kind="ExternalOutput").ap()
    # DRAM scratch used to broadcast softmax reciprocal rows across
    # partitions (DMA from DRAM may use a 0-step partition dim; SBUF may not)
    rcd = nc.dram_tensor("rcd", [8, RSP, RSF], BF16, kind="Internal").ap()

    with tile.TileContext(nc) as tc:
        with tc.tile_pool(name="wts", bufs=1) as wts, \
             tc.tile_pool(name="xtp", bufs=1) as xtp, \
             tc.tile_pool(name="qkv", bufs=1) as qkv, \
             tc.tile_pool(name="otp", bufs=1) as otp, \
             tc.tile_pool(name="ptp", bufs=2) as ptp, \
             tc.tile_pool(name="asb", bufs=2) as asbp, \
             tc.tile_pool(name="rsp", bufs=2) as rsp, \
             tc.tile_pool(name="bcp", bufs=2) as bcp, \
             tc.tile_pool(name="tmp", bufs=2) as tmpp, \
             tc.tile_pool(name="ysb", bufs=4) as ysbp, \
             tc.tile_pool(name="mkp", bufs=2) as mkp, \
             tc.tile_pool(name="stp", bufs=2, space="PSUM") as stp, \
             tc.tile_pool(name="avp", bufs=2, space="PSUM") as avp, \
             tc.tile_pool(name="mmp", bufs=2, space="PSUM") as mmp:

            # Pre-load the ACT table set containing Exp+Ln so the first use
            # doesn't pay a table load mid-pipeline.
            from concourse.hw_specs import get_activation_tables
            tables = get_activation_tables(nc.m.arch)
            combined_id = list(tables).index("natural_log_exp_and_others")
            nc.scalar.add_instruction(mybir.InstLoadActFuncSet(
                name=nc.get_next_instruction_name(), ins=[], outs=[],
                act_func_set_id=combined_id))

            # ---- input DMAs ----
            # sync ring: the phase-A-critical stream (wqkv[c], xt[c] pairs)
            wqkv = [wts.tile([P, 3 * DD], BF16, name=f"wqkv{c}")
                    for c in range(CCH)]
            xt = [xtp.tile([P, T], BF16, name=f"xt{c}") for c in range(CCH)]
            for c in range(CCH):
                nc.sync.dma_start(out=wqkv[c],
                                  in_=wqkvT[c * P:(c + 1) * P, :])
                nc.sync.dma_start(out=xt[c], in_=xT[c * P:(c + 1) * P, :])
            wk = [wqkv[c][:, 0:DD] for c in range(CCH)]
            wq = [wqkv[c][:, DD:2 * DD] for c in range(CCH)]
            wv = [wqkv[c][:, 2 * DD:3 * DD] for c in range(CCH)]
            # scalar ring: everything that isn't needed in the first ~15us
            bqk_t = wts.tile([P, 4], F32, name="bqk_t")
            nc.scalar.dma_start(out=bqk_t, in_=bqk)
            band_t = None
            if band_in is not None:
                band_t = wts.tile([P, P], F32, name="band_t")
                nc.scalar.dma_start(out=band_t, in_=band_in)
            if kbias_in is not None:
                kbias_t = wts.tile([P, NT], F32, name="kbias_t")
                nc.scalar.dma_start(out=kbias_t, in_=kbias_in)
            bv_sb = wts.tile([P, DD], F32, name="bv_t")
            nc.scalar.dma_start(out=bv_sb, in_=bv_in)
            wp = [wts.tile([P, C], BF16, name=f"wp{i}") for i in range(2)]
            for i in range(2):
                nc.scalar.dma_start(out=wp[i], in_=wpT[i * P:(i + 1) * P, :])

            qt = [qkv.tile([P, T], BF16, name=f"qt{i}") for i in range(2)]
            kt = [qkv.tile([P, T], BF16, name=f"kt{i}") for i in range(2)]
            vaug = [qkv.tile([P, NH * (D + 1)], BF16, name=f"vaug{j}")
                    for j in range(NT)]
            ot = [otp.tile([P, T], BF16, name=f"ot{i}") for i in range(2)]

            # ones columns (softmax denominator rows) written once, on the
            # otherwise-idle GPSIMD engine
            for j in range(NT):
                vview = vaug[j].rearrange("p (h x) -> p h x", h=NH)
                nc.gpsimd.memset(vview[:, :, D:D + 1], 1.0)

            # ---- phase A: kt[0]/qt[0], c-outer over 8 parallel chains ----
            # (each xT chunk arrival unblocks one matmul per chain; the PE
            # is never head-of-line blocked behind a not-yet-arrived chunk)
            stA = [stp.tile([P, 2 * TG], F32, name="st", tag="st")
                   for _ in range(2)]
            avA = [avp.tile([P, TG], F32, name="av", tag="av")
                   for _ in range(2)]
            mmA = [mmp.tile([P, TG], F32, name="mm", tag="mm")
                   for _ in range(2)]
            ps8 = [stA[0][:, 0:TG], stA[0][:, TG:2 * TG],
                   stA[1][:, 0:TG], stA[1][:, TG:2 * TG],
                   avA[0], avA[1], mmA[0], mmA[1]]
            specs = [(kt[0], wk, 2, tg) for tg in range(NG)] + \
                    [(qt[0], wq, 0, tg) for tg in range(NG)]
            for c in range(CCH):
                for idx, (dst, w, col, tg) in enumerate(specs):
                    nc.tensor.matmul(
                        ps8[idx],
                        lhsT=(w[c][:, 0:P]),
                        rhs=(xt[c][:, tg * TG:(tg + 1) * TG]),
                        start=(c == 0), stop=(c == CCH - 1))
            # drain order: the chains attn(0,3) needs first come first
            for idx in (0, 7, 1, 2, 3, 4, 5, 6):
                dst, w, col, tg = specs[idx]
                nc.vector.tensor_scalar_add(
                    dst[:, tg * TG:(tg + 1) * TG], ps8[idx],
                    bqk_t[:, col:col + 1])

            # ---- filler units (independent PE work interleaved into the
            # attention slots to cover the S->exp->AV latency) ----
            def v_proj(j):
                ps = mmp.tile([P, TG], F32, name="mm", tag="mm")
                for c in range(CCH):
                    nc.tensor.matmul(
                        ps[:, :DD],
                        lhsT=(xt[c][:, j * P:(j + 1) * P]),
                        rhs=(wv[c]),
                        start=(c == 0), stop=(c == CCH - 1))
                vview = vaug[j].rearrange("p (h x) -> p h x", h=NH)
                nc.vector.tensor_add(
                    vview[:, :, 0:D],
                    ps[:, :DD].rearrange("p (h x) -> p h x", h=NH),
                    bv_sb.rearrange("p (h x) -> p h x", h=NH))

            def qk1_chain(dst, w, col, tg):
                ps = mmp.tile([P, TG], F32, name="mm", tag="mm")
                for c in range(CCH):
                    nc.tensor.matmul(
                        ps,
                        lhsT=(w[c][:, P:2 * P]),
                        rhs=(xt[c][:, tg * TG:(tg + 1) * TG]),
                        start=(c == 0), stop=(c == CCH - 1))
                nc.vector.tensor_scalar_add(
                    dst[:, tg * TG:(tg + 1) * TG], ps, bqk_t[:, col:col + 1])

            def proj_unit(tt, tail=False):
                # both ec halves of one t-row: 2 matmul pairs, 2 casts, ONE
                # [128,1024] output DMA (fewer, bigger transfers keep the
                # sync ring clear for the latency-critical softmax bounces)
                ysb = ysbp.tile([P, 2 * TG], BF16, name="ysb", tag="ysb")
                for ec in range(2):
                    ps = mmp.tile([P, TG], F32, name="mm", tag="mm")
                    for i in range(2):
                        nc.tensor.matmul(
                            ps,
                            lhsT=(ot[i][:, tt * P:(tt + 1) * P]),
                            rhs=(wp[i][:, ec * TG:(ec + 1) * TG]),
                            start=(i == 0), stop=(i == 1))
                    if tail and ec == 1:
                        nc.scalar.activation(
                            ysb[:, ec * TG:(ec + 1) * TG], ps,
                            mybir.ActivationFunctionType.Identity)
                    else:
                        nc.vector.tensor_copy(
                            ysb[:, ec * TG:(ec + 1) * TG], ps)
                eng = nc.scalar if tail else nc.sync
                eng.dma_start(
                    out=yp[tt * P:(tt + 1) * P, :], in_=ysb)

            fq = []
            credit = [0.0]

            def pop_f(rate=1.0):
                credit[0] += rate
                while credit[0] >= 1.0:
                    credit[0] -= 1.0
                    if fq:
                        fq.pop(0)()

            # ---- attention ----
            def attn_block(i, g, rate, prev_epi):
                nj = NT if general_mask else 4 * g + 4
                av = [avp.tile([P, TG], F32, name="av", tag="av")
                      for _ in range(2)]

                def emit_av(j, trim, pt):
                    for h in range(2):
                        nc.tensor.matmul(
                            av[h][0:D + 1, trim:TG],
                            lhsT=(vaug[j][:, (2 * i + h) * (D + 1):
                                               (2 * i + h + 1) * (D + 1)]),
                            rhs=(pt[:, h * TG + trim:(h + 1) * TG]),
                            start=(j == 0), stop=(j == nj - 1),
                            skip_group_check=True)

                pend = None
                for j in range(nj):
                    r = j - 4 * g
                    trim = r * P if (r > 0 and not general_mask) else 0
                    st = stp.tile([P, 2 * TG], F32, name="st", tag="st")
                    for h in range(2):
                        nc.tensor.matmul(
                            st[:, h * TG + trim:(h + 1) * TG],
                            lhsT=(kt[i][64 * h:64 * h + 64,
                                             j * P:(j + 1) * P]),
                            rhs=(qt[i][64 * h:64 * h + 64,
                                            g * TG + trim:(g + 1) * TG]),
                            start=True, stop=True,
                            tile_position=(64 * h, 0))
                    if general_mask:
                        mk = mkp.tile([P, TG], F32, name="mk", tag="mk")
                        nc.sync.dma_start(
                            out=mk,
                            in_=maskT[j * P:(j + 1) * P, g * TG:(g + 1) * TG])
                        for h in range(2):
                            nc.vector.tensor_add(
                                st[:, h * TG:(h + 1) * TG],
                                st[:, h * TG:(h + 1) * TG], mk)
                    elif r >= 0:
                        for h in range(2):
                            sl = slice(h * TG + r * P, h * TG + (r + 1) * P)
                            nc.vector.tensor_add(st[:, sl], st[:, sl], band_t)
                    pt = ptp.tile([P, 2 * TG], BF16, name="pt", tag="pt")
                    kb = kbias_t[:, j:j + 1] if apply_kbias else 0.0
                    if trim > 0:
                        for h in range(2):
                            nc.scalar.activation(
                                pt[:, h * TG + trim:(h + 1) * TG],
                                st[:, h * TG + trim:(h + 1) * TG],
                                EXP, bias=kb, scale=SCL)
                    else:
                        nc.scalar.activation(pt, st, EXP, bias=kb, scale=SCL)
                    if pend is not None:
                        emit_av(*pend)
                    pend = (j, trim, pt)
                    if j == 1 and prev_epi is not None:
                        prev_epi()
                        prev_epi = None
                    pop_f(rate)
                emit_av(*pend)
                if prev_epi is not None:  # nj==1 can't happen, but be safe
                    prev_epi()

                # epilogue part a (now): free the PSUM accumulator banks and
                # kick off the denominator respread. Part b (deferred into
                # the next block) does the scalar Ln/Exp + bounce + muls so
                # the exp stream never idles waiting on this block's AVs.
                slot = i * NG + g
                asb = asbp.tile([D + 1, 2 * TG], F32, name="asb", tag="asb")
                nc.vector.tensor_copy(asb[:, 0:TG], av[0][0:D + 1, :])
                nc.vector.tensor_copy(asb[:, TG:2 * TG], av[1][0:D + 1, :])
                rs = rsp.tile([RSP, RSF], F32, name="rs", tag="rs")
                nc.sync.dma_start(out=rs, in_=asb[D:D + 1, :])

                def epi_b():
                    rsb = rsp.tile([RSP, RSF], BF16, name="rsb", tag="rsb")
                    nc.scalar.activation(rs, rs, LN)
                    nc.scalar.activation(rsb, rs, EXP, scale=-1.0)
                    nc.sync.dma_start(out=rcd[slot], in_=rsb)
                    bc = bcp.tile([P, 2 * TG], BF16, name="bc", tag="bc")
                    bcast_src = bass.AP(
                        tensor=rcd.tensor, offset=rcd[slot].offset,
                        ap=[[0, D], [1, 2 * TG]])
                    nc.sync.dma_start(out=bc[0:D, :], in_=bcast_src)
                    nc.vector.tensor_mul(
                        ot[i][0:D, g * TG:(g + 1) * TG],
                        asb[0:D, 0:TG], bc[0:D, 0:TG])
                    tm = tmpp.tile([P, TG], BF16, name="tm", tag="tm")
                    nc.vector.tensor_mul(tm[0:D, :], asb[0:D, TG:2 * TG],
                                         bc[0:D, TG:2 * TG])
                    nc.sync.dma_start(
                        out=ot[i][64:128, g * TG:(g + 1) * TG],
                        in_=tm[0:D, :])

                return epi_b

            # ---- schedule ----
            for j in range(NT):
                fq.append(partial(v_proj, j))
            for tg in range(NG):
                fq.append(partial(qk1_chain, kt[1], wk, 3, tg))
            fq.append(partial(qk1_chain, qt[1], wq, 1, 3))
            for tg in range(3):
                fq.append(partial(qk1_chain, qt[1], wq, 1, tg))

            def with_proj(epi_b, g):
                # proj units of group g may only enter the filler queue once
                # the epilogue writing ot[*] for g has actually been EMITTED
                # (emission order defines the dependency graph)
                def f():
                    epi_b()
                    for tt in range(4 * g, 4 * g + 4):
                        fq.append(partial(proj_unit, tt))
                return f

            pop_f(4.0)  # v_proj 0..3 ahead of attn(0,3)'s first AVs
            epi = attn_block(0, 3, 1.0, None)
            epi = attn_block(0, 2, 1.0, epi)
            epi = attn_block(1, 3, 1.0, epi)
            epi = attn_block(1, 2, 0.4, with_proj(epi, 3))
            epi = attn_block(0, 1, 0.3, epi)
            epi = attn_block(1, 1, 0.3, with_proj(epi, 2))
            epi = attn_block(0, 0, 0.5, epi)
            epi = attn_block(1, 0, 0.5, with_proj(epi, 1))
            epi()
            for tt in range(4):
                proj_unit(tt, tail=True)
            while fq:
                pop_f(1.0)

    nc.compile()
    return nc


@lru_cache(maxsize=4)
def _program(apply_kbias: bool, general_mask: bool) -> bass.Bass:
    return build_program(apply_kbias, general_mask)


def _host_prep(inputs):
    x = np.asarray(inputs["x"], np.float32)
    Wq = np.asarray(inputs["Wq"], np.float32)
    bq = np.asarray(inputs["bq"], np.float32)
    Wk = np.asarray(inputs["Wk"], np.float32)
    bk = np.asarray(inputs["bk"], np.float32)
    Wv = np.asarray(inputs["Wv"], np.float32)
    bv = np.asarray(inputs["bv"], np.float32)
    Wp = np.asarray(inputs["Wp"], np.float32)
    attn_mask = np.asarray(inputs["attn_mask"])
    valid = np.asarray(inputs["valid_input_mask"])

    tril = np.tril(np.ones((T, T), attn_mask.dtype))
    causal = all(np.array_equal(attn_mask[b], tril) for b in range(B))
    # folded into the exp's bias (which applies after the 1/sqrt(d) scale)
    kbias_all = (valid.astype(np.float32) - 1.0) * 1e6
    apply_kbias = bool((valid == 0).any())

    band = np.where(np.arange(P)[:, None] <= np.arange(P)[None, :],
                    np.float32(0.0), np.float32(NEG))

    in_maps = []
    for core in range(NCORES):
        b, hg = divmod(core, 4)
        sl = slice(hg * DD, (hg + 1) * DD)
        wqkv = np.concatenate(
            [Wk[sl, :].T, Wq[sl, :].T, Wv[sl, :].T], axis=1)
        m = {
            "xT": np.ascontiguousarray(x[b].T).astype(ml_dtypes.bfloat16),
            "wqkvT": np.ascontiguousarray(wqkv).astype(ml_dtypes.bfloat16),
            "wpT": np.ascontiguousarray(Wp[:, sl].T).astype(ml_dtypes.bfloat16),
            "bqk": np.ascontiguousarray(
                np.stack([bq[sl][:P], bq[sl][P:], bk[sl][:P], bk[sl][P:]], 1)),
            "bv_sb": np.ascontiguousarray(np.tile(bv[sl], (P, 1))),
        }
        if apply_kbias:
            m["kbias"] = np.ascontiguousarray(kbias_all[b].reshape(NT, P).T)
        if not causal:
            m["maskT"] = np.ascontiguousarray(
                (attn_mask[b].T.astype(np.float32) - 1.0) * (-NEG))
        else:
            m["band"] = band
        in_maps.append(m)
    return in_maps, apply_kbias, causal


def _run(inputs, trace=False, trace_cores=None):
    global LAST_RESULTS
    in_maps, apply_kbias, causal = _host_prep(inputs)
    nc = _program(apply_kbias, not causal)
    res = bass_utils.run_bass_kernel_spmd(
        nc, in_maps, core_ids=list(range(NCORES)), trace=trace,
        trace_cores=trace_cores)
    LAST_RESULTS = res

    bp = np.asarray(inputs["bp"], np.float32)
    y = np.zeros((B, T, C), np.float32)
    for core in range(NCORES):
        y[core // 4] += np.asarray(res.results[core]["yp"], np.float32)
    y += bp[None, None, :]
    return y


def kernel(**inputs) -> np.ndarray:
    return _run(inputs)


# revision 13
# speedup vs baseline: 1.0989x; 1.0555x over previous
"""Causal self-attention (B=2, T=2048, C=1024, H=16) on 8 TRN2 NeuronCores.

Sharding: core = (batch b, head-group hg) with b in {0,1}, hg in {0..3};
each core computes Q/K/V projections and attention for its 4 heads on its
batch, plus the row-parallel slice of the output projection. The host sums
the 4 per-core partial projections per batch and adds the output bias.

v3 layout (all transposed so softmax needs no on-chip transposes):
  - Q^T, K^T [dd, t] via 8 parallel c-outer PSUM chains (no head-of-line
    blocking on the streaming xT DMA); V [t, dd] via per-t-chunk chains.
    QKV weights arrive packed as one [C, 3*DD] tensor (one DMA per chunk).
  - S^T[s, t]: 2 heads row-packed (K=64 at row offsets 0/64, concurrent
    streams); diagonal blocks trimmed to cols >= r*P; causal masking via an
    additive 128x128 band.
  - exp on ScalarE straight out of PSUM (scale + optional kbias folded in).
  - softmax denominators from an all-ones column appended to V (M=65 AV
    matmul); normalization deferred to after AV.
  - AV emission runs one j behind S so the PE never head-of-line blocks on
    the exp; independent filler work (V chains, second-half QK chains,
    output-projection tiles) is interleaved between attention slots to keep
    the PE dense (max p-state = 2.4 GHz needs ~3us of continuous work).
  - epilogue per (i,g): PSUM->SBUF copies + a DMA that respreads the
    denominator row to [8,256] (at block end), then deferred into the next
    block: Ln+Exp(-x) on [8,256] (cheap: ACT cost tracks free size), DRAM
    bounce broadcast in bf16, two DVE muls. ScalarE's exp stream never
    waits on an epilogue.
  - y_partial[t, e] accumulated over the 2 dd-chunks, written back as bf16
    on the GPSIMD software-DGE queue (keeps the latency-critical softmax
    bounce DMAs alone on the sync ring); host sums partials in f32.
"""

import math
from functools import lru_cache, partial

import ml_dtypes
import numpy as np

import concourse.bass as bass
import concourse.mybir as mybir
from concourse import bacc
import concourse.tile as tile
from concourse import bass_utils

F32 = mybir.dt.float32
BF16 = mybir.dt.bfloat16
EXP = mybir.ActivationFunctionType.Exp
LN = mybir.ActivationFunctionType.Ln

B, T, C, H = 2, 2048, 1024, 16
NCORES = 8
NH = 4            # heads per core
D = C // H        # 64
DD = NH * D       # 256 channels per core
P = 128
TG = 512          # t-group width (matmul moving dim)
NG = T // TG      # 4
NT = T // P       # 16 s-chunks
CCH = C // P      # 8 contraction chunks
NEG = -8.0e6      # pre-scale additive mask; *0.125 = -1e6 like the reference
SCL = 1.0 / math.sqrt(D)
RSP, RSF = 8, (2 * TG) // 8   # denominator respread shape [8, 256]

LAST_RESULTS = None  # BassKernelResults of the most recent run (for test.py)


def build_program(apply_kbias: bool, general_mask: bool) -> bass.Bass:
    nc = bacc.Bacc("TRN2", target_bir_lowering=False, debug=False,
                   enable_asserts=False)

    xT = nc.dram_tensor("xT", [C, T], BF16, kind="ExternalInput").ap()
    wqkvT = nc.dram_tensor("wqkvT", [C, 3 * DD], BF16, kind="ExternalInput").ap()
    wpT = nc.dram_tensor("wpT", [DD, C], BF16, kind="ExternalInput").ap()
    bqk = nc.dram_tensor("bqk", [P, 4], F32, kind="ExternalInput").ap()
    bv_in = nc.dram_tensor("bv_sb", [P, DD], F32, kind="ExternalInput").ap()
    kbias_in = None
    if apply_kbias:
        kbias_in = nc.dram_tensor("kbias", [P, NT], F32, kind="ExternalInput").ap()
    band_in = maskT = None
    if general_mask:
        maskT = nc.dram_tensor("maskT", [T, T], F32, kind="ExternalInput").ap()
    else:
        band_in = nc.dram_tensor("band", [P, P], F32, kind="ExternalInput").ap()
    yp = nc.dram_tensor("yp", [T, C], BF16, kind="ExternalOutput").ap()
    # DRAM scratch used to broadcast softmax reciprocal rows across
    # partitions (DMA from DRAM may use a 0-step partition dim; SBUF may not)
    rcd = nc.dram_tensor("rcd", [8, RSP, RSF], BF16, kind="Internal").ap()

    with tile.TileContext(nc) as tc:
        with tc.tile_pool(name="wts", bufs=1) as wts, \
             tc.tile_pool(name="xtp", bufs=1) as xtp, \
             tc.tile_pool(name="qkv", bufs=1) as qkv, \
             tc.tile_pool(name="otp", bufs=1) as otp, \
             tc.tile_pool(name="ptp", bufs=2) as ptp, \
             tc.tile_pool(name="asb", bufs=2) as asbp, \
             tc.tile_pool(name="rsp", bufs=2) as rsp, \
             tc.tile_pool(name="bcp", bufs=2) as bcp, \
             tc.tile_pool(name="tmp", bufs=2) as tmpp, \
             tc.tile_pool(name="ysb", bufs=4) as ysbp, \
             tc.tile_pool(name="mkp", bufs=2) as mkp, \
             tc.tile_pool(name="stp", bufs=2, space="PSUM") as stp, \
             tc.tile_pool(name="avp", bufs=2, space="PSUM") as avp, \
             tc.tile_pool(name="mmp", bufs=2, space="PSUM") as mmp:

            # Pre-load the ACT table set containing Exp+Ln so the first use
            # doesn't pay a table load mid-pipeline.
            from concourse.hw_specs import get_activation_tables
            tables = get_activation_tables(nc.m.arch)
            combined_id = list(tables).index("natural_log_exp_and_others")
            nc.scalar.add_instruction(mybir.InstLoadActFuncSet(
                name=nc.get_next_instruction_name(), ins=[], outs=[],
                act_func_set_id=combined_id))

            # ---- input DMAs ----
            # sync ring: the phase-A-critical stream (wqkv[c], xt[c] pairs)
            wqkv = [wts.tile([P, 3 * DD], BF16, name=f"wqkv{c}")
                    for c in range(CCH)]
            xt = [xtp.tile([P, T], BF16, name=f"xt{c}") for c in range(CCH)]
            for c in range(CCH):
                nc.sync.dma_start(out=wqkv[c],
                                  in_=wqkvT[c * P:(c + 1) * P, :])
                nc.sync.dma_start(out=xt[c], in_=xT[c * P:(c + 1) * P, :])
            wk = [wqkv[c][:, 0:DD] for c in range(CCH)]
            wq = [wqkv[c][:, DD:2 * DD] for c in range(CCH)]
            wv = [wqkv[c][:, 2 * DD:3 * DD] for c in range(CCH)]
            # scalar ring: everything that isn't needed in the first ~15us
            bqk_t = wts.tile([P, 4], F32, name="bqk_t")
            nc.scalar.dma_start(out=bqk_t, in_=bqk)
            band_t = None
            if band_in is not None:
                band_t = wts.tile([P, P], F32, name="band_t")
                nc.scalar.dma_start(out=band_t, in_=band_in)
            if kbias_in is not None:
                kbias_t = wts.tile([P, NT], F32, name="kbias_t")
                nc.scalar.dma_start(out=kbias_t, in_=kbias_in)
            bv_sb = wts.tile([P, DD], F32, name="bv_t")
            nc.scalar.dma_start(out=bv_sb, in_=bv_in)
            wp = [wts.tile([P, C], BF16, name=f"wp{i}") for i in range(2)]
            for i in range(2):
                nc.scalar.dma_start(out=wp[i], in_=wpT[i * P:(i + 1) * P, :])

            qt = [qkv.tile([P, T], BF16, name=f"qt{i}") for i in range(2)]
            kt = [qkv.tile([P, T], BF16, name=f"kt{i}") for i in range(2)]
            vaug = [qkv.tile([P, NH * (D + 1)], BF16, name=f"vaug{j}")
                    for j in range(NT)]
            ot = [otp.tile([P, T], BF16, name=f"ot{i}") for i in range(2)]

            # ones columns (softmax denominator rows) written once, on the
            # otherwise-idle GPSIMD engine
            for j in range(NT):
                vview = vaug[j].rearrange("p (h x) -> p h x", h=NH)
                nc.gpsimd.memset(vview[:, :, D:D + 1], 1.0)

            # ---- phase A: kt[0]/qt[0], c-outer over 8 parallel chains ----
            # (each xT chunk arrival unblocks one matmul per chain; the PE
            # is never head-of-line blocked behind a not-yet-arrived chunk)
            stA = [stp.tile([P, 2 * TG], F32, name="st", tag="st")
                   for _ in range(2)]
            avA = [avp.tile([P, TG], F32, name="av", tag="av")
                   for _ in range(2)]
            mmA = [mmp.tile([P, TG], F32, name="mm", tag="mm")
                   for _ in range(2)]
            ps8 = [stA[0][:, 0:TG], stA[0][:, TG:2 * TG],
                   stA[1][:, 0:TG], stA[1][:, TG:2 * TG],
                   avA[0], avA[1], mmA[0], mmA[1]]
            specs = [(kt[0], wk, 2, tg) for tg in range(NG)] + \
                    [(qt[0], wq, 0, tg) for tg in range(NG)]
            for c in range(CCH):
                for idx, (dst, w, col, tg) in enumerate(specs):
                    nc.tensor.matmul(
                        ps8[idx],
                        lhsT=(w[c][:, 0:P]),
                        rhs=(xt[c][:, tg * TG:(tg + 1) * TG]),
                        start=(c == 0), stop=(c == CCH - 1))
            # drain order: the chains attn(0,3) needs first come first
            for idx in (0, 7, 1, 2, 3, 4, 5, 6):
                dst, w, col, tg = specs[idx]
                nc.vector.tensor_scalar_add(
                    dst[:, tg * TG:(tg + 1) * TG], ps8[idx],
                    bqk_t[:, col:col + 1])

            # ---- filler units (independent PE work interleaved into the
            # attention slots to cover the S->exp->AV latency) ----
            def v_proj(j):
                ps = mmp.tile([P, TG], F32, name="mm", tag="mm")
                for c in range(CCH):
                    nc.tensor.matmul(
                        ps[:, :DD],
                        lhsT=(xt[c][:, j * P:(j + 1) * P]),
                        rhs=(wv[c]),
                        start=(c == 0), stop=(c == CCH - 1))
                vview = vaug[j].rearrange("p (h x) -> p h x", h=NH)
                nc.vector.tensor_add(
                    vview[:, :, 0:D],
                    ps[:, :DD].rearrange("p (h x) -> p h x", h=NH),
                    bv_sb.rearrange("p (h x) -> p h x", h=NH))

            def qk1_chain(dst, w, col, tg):
                ps = mmp.tile([P, TG], F32, name="mm", tag="mm")
                for c in range(CCH):
                    nc.tensor.matmul(
                        ps,
                        lhsT=(w[c][:, P:2 * P]),
                        rhs=(xt[c][:, tg * TG:(tg + 1) * TG]),
                        start=(c == 0), stop=(c == CCH - 1))
                nc.vector.tensor_scalar_add(
                    dst[:, tg * TG:(tg + 1) * TG], ps, bqk_t[:, col:col + 1])

            def proj_unit(tt, tail=False):
                # both ec halves of one t-row: 2 matmul pairs, 2 casts, ONE
                # [128,1024] output DMA (fewer, bigger transfers keep the
                # sync ring clear for the latency-critical softmax bounces)
                ysb = ysbp.tile([P, 2 * TG], BF16, name="ysb", tag="ysb")
                for ec in range(2):
                    ps = mmp.tile([P, TG], F32, name="mm", tag="mm")
                    for i in range(2):
                        nc.tensor.matmul(
                            ps,
                            lhsT=(ot[i][:, tt * P:(tt + 1) * P]),
                            rhs=(wp[i][:, ec * TG:(ec + 1) * TG]),
                            start=(i == 0), stop=(i == 1))
                    if tail and ec == 1:
                        nc.scalar.activation(
                            ysb[:, ec * TG:(ec + 1) * TG], ps,
                            mybir.ActivationFunctionType.Identity)
                    else:
                        nc.vector.tensor_copy(
                            ysb[:, ec * TG:(ec + 1) * TG], ps)
                eng = nc.scalar if tail else nc.sync
                eng.dma_start(
                    out=yp[tt * P:(tt + 1) * P, :], in_=ysb)

            fq = []
            credit = [0.0]

            def pop_f(rate=1.0):
                credit[0] += rate
                while credit[0] >= 1.0:
                    credit[0] -= 1.0
                    if fq:
                        fq.pop(0)()

            # ---- attention ----
            def attn_block(i, g, rate, prev_epi, fast_epi=False):
                nj = NT if general_mask else 4 * g + 4
                av = [avp.tile([P, TG], F32, name="av", tag="av")
                      for _ in range(2)]

                def emit_av(j, trim, pt):
                    for h in range(2):
                        nc.tensor.matmul(
                            av[h][0:D + 1, trim:TG],
                            lhsT=(vaug[j][:, (2 * i + h) * (D + 1):
                                               (2 * i + h + 1) * (D + 1)]),
                            rhs=(pt[:, h * TG + trim:(h + 1) * TG]),
                            start=(j == 0), stop=(j == nj - 1),
                            skip_group_check=True)

                pend = None
                for j in range(nj):
                    r = j - 4 * g
                    trim = r * P if (r > 0 and not general_mask) else 0
                    st = stp.tile([P, 2 * TG], F32, name="st", tag="st")
                    for h in range(2):
                        nc.tensor.matmul(
                            st[:, h * TG + trim:(h + 1) * TG],
                            lhsT=(kt[i][64 * h:64 * h + 64,
                                             j * P:(j + 1) * P]),
                            rhs=(qt[i][64 * h:64 * h + 64,
                                            g * TG + trim:(g + 1) * TG]),
                            start=True, stop=True,
                            tile_position=(64 * h, 0))
                    if general_mask:
                        mk = mkp.tile([P, TG], F32, name="mk", tag="mk")
                        nc.sync.dma_start(
                            out=mk,
                            in_=maskT[j * P:(j + 1) * P, g * TG:(g + 1) * TG])
                        for h in range(2):
                            nc.vector.tensor_add(
                                st[:, h * TG:(h + 1) * TG],
                                st[:, h * TG:(h + 1) * TG], mk)
                    elif r >= 0:
                        for h in range(2):
                            sl = slice(h * TG + r * P, h * TG + (r + 1) * P)
                            nc.vector.tensor_add(st[:, sl], st[:, sl], band_t)
                    pt = ptp.tile([P, 2 * TG], BF16, name="pt", tag="pt")
                    kb = kbias_t[:, j:j + 1] if apply_kbias else 0.0
                    if trim > 0:
                        for h in range(2):
                            nc.scalar.activation(
                                pt[:, h * TG + trim:(h + 1) * TG],
                                st[:, h * TG + trim:(h + 1) * TG],
                                EXP, bias=kb, scale=SCL)
                    else:
                        nc.scalar.activation(pt, st, EXP, bias=kb, scale=SCL)
                    if pend is not None:
                        emit_av(*pend)
                    pend = (j, trim, pt)
                    if j == 1 and prev_epi is not None:
                        prev_epi()
                        prev_epi = None
                    pop_f(rate)
                emit_av(*pend)
                if prev_epi is not None:  # nj==1 can't happen, but be safe
                    prev_epi()

                # epilogue part a (now): free the PSUM accumulator banks and
                # kick off the denominator respread. Part b (deferred into
                # the next block) does the scalar Ln/Exp + bounce + muls so
                # the exp stream never idles waiting on this block's AVs.
                slot = i * NG + g
                ring = nc.scalar if fast_epi else nc.sync
                asb = asbp.tile([D + 1, 2 * TG], F32, name="asb", tag="asb")
                nc.vector.tensor_copy(asb[:, 0:TG], av[0][0:D + 1, :])
                if fast_epi:  # scalar engine is idle at the very end
                    nc.scalar.activation(
                        asb[:, TG:2 * TG], av[1][0:D + 1, :],
                        mybir.ActivationFunctionType.Identity)
                else:
                    nc.vector.tensor_copy(asb[:, TG:2 * TG], av[1][0:D + 1, :])
                rs = rsp.tile([RSP, RSF], F32, name="rs", tag="rs")
                ring.dma_start(out=rs, in_=asb[D:D + 1, :])

                def epi_b():
                    rsb = rsp.tile([RSP, RSF], BF16, name="rsb", tag="rsb")
                    nc.scalar.activation(rs, rs, LN)
                    nc.scalar.activation(rsb, rs, EXP, scale=-1.0)
                    ring.dma_start(out=rcd[slot], in_=rsb)
                    bc = bcp.tile([P, 2 * TG], BF16, name="bc", tag="bc")
                    bcast_src = bass.AP(
                        tensor=rcd.tensor, offset=rcd[slot].offset,
                        ap=[[0, D], [1, 2 * TG]])
                    ring.dma_start(out=bc[0:D, :], in_=bcast_src)
                    # upper-half (partition-shifted via DMA) first so it
                    # lands while the lower-half mul still runs
                    tm = tmpp.tile([P, TG], BF16, name="tm", tag="tm")
                    nc.vector.tensor_mul(tm[0:D, :], asb[0:D, TG:2 * TG],
                                         bc[0:D, TG:2 * TG])
                    ring.dma_start(
                        out=ot[i][64:128, g * TG:(g + 1) * TG],
                        in_=tm[0:D, :])
                    nc.vector.tensor_mul(
                        ot[i][0:D, g * TG:(g + 1) * TG],
                        asb[0:D, 0:TG], bc[0:D, 0:TG])

                return epi_b

            # ---- schedule ----
            for j in range(NT):
                fq.append(partial(v_proj, j))
            for tg in range(NG):
                fq.append(partial(qk1_chain, kt[1], wk, 3, tg))
            fq.append(partial(qk1_chain, qt[1], wq, 1, 3))
            for tg in range(3):
                fq.append(partial(qk1_chain, qt[1], wq, 1, tg))

            def with_proj(epi_b, g):
                # proj units of group g may only enter the filler queue once
                # the epilogue writing ot[*] for g has actually been EMITTED
                # (emission order defines the dependency graph)
                def f():
                    epi_b()
                    for tt in range(4 * g, 4 * g + 4):
                        fq.append(partial(proj_unit, tt))
                return f

            pop_f(4.0)  # v_proj 0..3 ahead of attn(0,3)'s first AVs
            epi = attn_block(0, 3, 1.0, None)
            epi = attn_block(0, 2, 1.0, epi)
            epi = attn_block(1, 3, 1.0, epi)
            epi = attn_block(1, 2, 0.4, with_proj(epi, 3))
            epi = attn_block(0, 1, 0.3, with_proj(epi, 2))
            epi = attn_block(1, 1, 0.3, epi)
            epi = attn_block(0, 0, 0.6, with_proj(epi, 1))
            epi = attn_block(1, 0, 0.6, epi, fast_epi=True)
            epi()
            while fq:
                pop_f(1.0)
            for tt in range(4):
                proj_unit(tt, tail=True)

    nc.compile()
    return nc


@lru_cache(maxsize=4)
def _program(apply_kbias: bool, general_mask: bool) -> bass.Bass:
    return build_program(apply_kbias, general_mask)


def _host_prep(inputs):
    x = np.asarray(inputs["x"], np.float32)
    Wq = np.asarray(inputs["Wq"], np.float32)
    bq = np.asarray(inputs["bq"], np.float32)
    Wk = np.asarray(inputs["Wk"], np.float32)
    bk = np.asarray(inputs["bk"], np.float32)
    Wv = np.asarray(inputs["Wv"], np.float32)
    bv = np.asarray(inputs["bv"], np.float32)
    Wp = np.asarray(inputs["Wp"], np.float32)
    attn_mask = np.asarray(inputs["attn_mask"])
    valid = np.asarray(inputs["valid_input_mask"])

    tril = np.tril(np.ones((T, T), attn_mask.dtype))
    causal = all(np.array_equal(attn_mask[b], tril) for b in range(B))
    # folded into the exp's bias (which applies after the 1/sqrt(d) scale)
    kbias_all = (valid.astype(np.float32) - 1.0) * 1e6
    apply_kbias = bool((valid == 0).any())

    band = np.where(np.arange(P)[:, None] <= np.arange(P)[None, :],
                    np.float32(0.0), np.float32(NEG))

    in_maps = []
    for core in range(NCORES):
        b, hg = divmod(core, 4)
        sl = slice(hg * DD, (hg + 1) * DD)
        wqkv = np.concatenate(
            [Wk[sl, :].T, Wq[sl, :].T, Wv[sl, :].T], axis=1)
        m = {
            "xT": np.ascontiguousarray(x[b].T).astype(ml_dtypes.bfloat16),
            "wqkvT": np.ascontiguousarray(wqkv).astype(ml_dtypes.bfloat16),
            "wpT": np.ascontiguousarray(Wp[:, sl].T).astype(ml_dtypes.bfloat16),
            "bqk": np.ascontiguousarray(
                np.stack([bq[sl][:P], bq[sl][P:], bk[sl][:P], bk[sl][P:]], 1)),
            "bv_sb": np.ascontiguousarray(np.tile(bv[sl], (P, 1))),
        }
        if apply_kbias:
            m["kbias"] = np.ascontiguousarray(kbias_all[b].reshape(NT, P).T)
        if not causal:
            m["maskT"] = np.ascontiguousarray(
                (attn_mask[b].T.astype(np.float32) - 1.0) * (-NEG))
        else:
            m["band"] = band
        in_maps.append(m)
    return in_maps, apply_kbias, causal


def _run(inputs, trace=False, trace_cores=None):
    global LAST_RESULTS
    in_maps, apply_kbias, causal = _host_prep(inputs)
    nc = _program(apply_kbias, not causal)
    res = bass_utils.run_bass_kernel_spmd(
        nc, in_maps, core_ids=list(range(NCORES)), trace=trace,
        trace_cores=trace_cores)
    LAST_RESULTS = res

    bp = np.asarray(inputs["bp"], np.float32)
    y = np.zeros((B, T, C), np.float32)
    for core in range(NCORES):
        y[core // 4] += np.asarray(res.results[core]["yp"], np.float32)
    y += bp[None, None, :]
    return y


def kernel(**inputs) -> np.ndarray:
    return _run(inputs)
